# revision 16
# baseline (speedup 1.0000x reference)
"""Trainium2 Bass kernel for windowed multi-head attention with relative
position bias (Swin-style block):

    qkv = x @ qkv_w.T + [q_bias, 0, v_bias]
    q, k, v = split(qkv);  q *= hd**-0.5
    attn = softmax(q @ k.T + rel_table[rel_index])
    out  = (attn @ v) @ proj_w.T + proj_b

Shapes: x [8, 32, 32, 768], 12 heads, head_dim 64, N=1024 tokens.

Sharding: pure data-parallel - one batch element per NeuronCore, 8 cores,
no collectives.

V2 design notes (vs the phase-separated baseline):
  - all matmuls fp16 (weights/x cast on host); psum accumulates f32.
  - single fully-interleaved pipeline: per head-pair j the S-matmul
    stream is interleaved with next pair's QK projection matmuls (and
    the last pair with proj tiles) so the PE queue never stalls waiting
    for the ACT exp to drain PSUM slots - this both removes bubbles and
    keeps the PE p-state ramped.
  - softmax denominators: ones-column in v_aug -> row 64 of the PV psum;
    reciprocal straight off PSUM into inv_sb (DVE), broadcast across
    partitions with gpsimd partition_broadcast (no PE selector matmuls,
    no DRAM scratch round-trip), and the normalization is fused into the
    PV psum eviction via scalar_tensor_tensor: attn = (po * 1) * inv_bc.
  - engine balance: ACT runs exp + the per-head ln/exp inverse (one
    preloaded {ln,exp} table set, no reloads); evictions + 6 of 8
    bias-multiplies on DVE, the last two k-tiles' multiplies on
    GPSIMD (pure-DVE overloads that queue and paces the PV fillers).
  - PSUM: pss [128,1024]x2 (4 banks) + po [65,2x512] (2) + shared
    qkv/proj pool [128,512]x2 (2) = 8 banks exactly.
"""

import numpy as np

_CACHE = {}

B = 8
WS = 32
N = WS * WS            # 1024 tokens
C = 768
NH = 12
HD = 64
P = 128
QC = 2                 # q chunks of 512
QN = N // QC           # 512
KT = N // P            # 8 k tiles
CT = C // P            # 6 contraction tiles
OT_QK = (2 * C) // P   # 12 output tiles for q,k rows
VC = 2                 # v output chunks of 384
VN = C // VC           # 384
NP = NH // 2           # 6 head pairs


def _build():
    import concourse.bass as bass
    import concourse.bacc as bacc
    import concourse.mybir as mybir
    import concourse.tile as tile
    import itertools
    from concourse.bass import _add_dep_helper

    f32 = mybir.dt.float32
    f16 = mybir.dt.float16
    AF = mybir.ActivationFunctionType
    MUL = mybir.AluOpType.mult

    nc = bacc.Bacc(None, target_bir_lowering=False)

    xT_d = nc.dram_tensor("xT", [C, N], f16, kind="ExternalInput")
    wqk_d = nc.dram_tensor("wqk", [C, 2 * C], f16, kind="ExternalInput")
    wv_d = nc.dram_tensor("wv", [C, C], f16, kind="ExternalInput")
    wp_d = nc.dram_tensor("wp", [C, C], f16, kind="ExternalInput")
    qkb_d = nc.dram_tensor("qkb", [OT_QK, P], f32, kind="ExternalInput")
    vb_d = nc.dram_tensor("vb", [C], f16, kind="ExternalInput")
    pb_d = nc.dram_tensor("pb", [CT, P], f32, kind="ExternalInput")
    biasT_d = nc.dram_tensor("biasT", [NH, N, N], f16, kind="ExternalInput")
    yT_d = nc.dram_tensor("yT", [C, N], f32, kind="ExternalOutput")
    inv_d = nc.dram_tensor("inv_scr", [NH, N], f32)

    with tile.TileContext(nc) as tc:
        with (
            tc.tile_pool(name="cst", bufs=1) as cst,
            tc.tile_pool(name="bias_pool", bufs=2) as bias_pool,
            tc.tile_pool(name="pt_pool", bufs=2) as pt_pool,
            tc.tile_pool(name="ps_s", bufs=2, space="PSUM") as ps_s,
            tc.tile_pool(name="ps_o", bufs=2, space="PSUM") as ps_o,
            tc.tile_pool(name="ps_w", bufs=2, space="PSUM") as ps_w,
        ):
            # ---- persistent SBUF ----
            q_t = cst.tile([P, CT, N], f16)          # Q^T  [c, t]
            k_pad = cst.tile([P, NH, N], f16)        # zero-padded K^T per head
            v_aug = cst.tile([P, KT, NH, HD + 1], f16)  # V + ones column
            attn = cst.tile([P, CT, N], f16)         # normalized attn out ^T
            xT = cst.tile([P, CT, N], f16)
            wv = cst.tile([P, CT, C], f16)
            wqk = cst.tile([P, CT, 2 * C], f16)
            wp = cst.tile([P, CT, C], f16)
            qkb = cst.tile([P, OT_QK], f32)
            vb_bc = cst.tile([P, C], f16)
            pbias = cst.tile([P, CT], f32)

            biasT = {}

            def load_bias(h):
                biasT[h] = bias_pool.tile([P, KT, N], f16, tag="biasT",
                                          name=f"biasT{h}")
                nc.sync.dma_start(
                    biasT[h], biasT_d[h].rearrange("(kt p) q -> p kt q", p=P))

            # ---- input DMAs, priority order ----
            xT_src = xT_d[:].rearrange("(k p) t -> p k t", p=P)
            wv_src = wv_d[:].rearrange("(k p) o -> p k o", p=P)
            wqk_src = wqk_d[:].rearrange("(k p) o -> p k o", p=P)
            wp_src = wp_d[:].rearrange("(k p) o -> p k o", p=P)
            for k in range(CT):
                nc.sync.dma_start(xT[:, k, :], xT_src[:, k, :])
                nc.sync.dma_start(wv[:, k, :], wv_src[:, k, :])
            nc.sync.dma_start(vb_bc, bass.AP(tensor=vb_d, offset=0,
                                             ap=[[0, P], [1, C]]))
            nc.sync.dma_start(qkb, qkb_d[:].rearrange("j p -> p j"))
            for k in range(CT):
                nc.sync.dma_start(wqk[:, k, :], wqk_src[:, k, :])
            load_bias(0)
            load_bias(1)
            for k in range(CT):
                nc.sync.dma_start(wp[:, k, :], wp_src[:, k, :])
            nc.sync.dma_start(pbias, pb_d[:].rearrange("j p -> p j"))

            # preload the {ln, exp} activation table set once so the
            # per-head inv = exp(-ln(s)) chain never thrashes ACT tables
            # (set 6 = natural_log_exp_and_others in act_info.json)
            nc.scalar.add_instruction(mybir.InstLoadActFuncSet(
                name="preload_ln_exp", act_func_set_id=6, ins=[], outs=[]))

            # one-time fills on GPSIMD (idle engine)
            nc.gpsimd.memset(k_pad[64:128, 0:NH:2, :], 0.0)
            nc.gpsimd.memset(k_pad[0:64, 1:NH:2, :], 0.0)
            nc.gpsimd.memset(v_aug[:, :, :, HD:HD + 1], 1.0)

            # ---- emission helpers ----
            def v_units(tt):
                # V natural [t, o']: lhsT = xT tile (shared across vc)
                pvs = [ps_w.tile([P, QN], f32, tag="pw", name=f"pv{vc}")
                       for vc in range(VC)]
                for k in range(CT):
                    for vc in range(VC):
                        nc.tensor.matmul(
                            pvs[vc][:, 0:VN], xT[:, k, tt * P:(tt + 1) * P],
                            wv[:, k, vc * VN:(vc + 1) * VN],
                            start=(k == 0), stop=(k == CT - 1))
                    yield
                for vc in range(VC):
                    h0 = vc * (NH // VC)
                    nc.vector.tensor_add(
                        v_aug[:, tt, h0:h0 + NH // VC, 0:HD],
                        pvs[vc][:, 0:VN], vb_bc[:, vc * VN:(vc + 1) * VN])
                yield

            def qk_units(j):
                # (jj, k) matmul units for the QK projection tile pair j
                for jj in (j, CT + j):
                    pqs = [ps_w.tile([P, QN], f32, tag="pw", name=f"pq{qc}")
                           for qc in range(QC)]
                    for k in range(CT):
                        for qc in range(QC):
                            nc.tensor.matmul(
                                pqs[qc], wqk[:, k, jj * P:(jj + 1) * P],
                                xT[:, k, qc * QN:(qc + 1) * QN],
                                start=(k == 0), stop=(k == CT - 1))
                        yield
                    # evictions on DVE with per-partition bias
                    for qc in range(QC):
                        if jj < CT:
                            nc.vector.tensor_scalar_add(
                                q_t[:, jj, qc * QN:(qc + 1) * QN], pqs[qc],
                                qkb[:, jj:jj + 1])
                        else:
                            h0 = 2 * (jj - CT)
                            nc.vector.tensor_scalar_add(
                                k_pad[0:64, h0, qc * QN:(qc + 1) * QN],
                                pqs[qc][0:64, :], qkb[0:64, jj:jj + 1])
                            nc.vector.tensor_scalar_add(
                                k_pad[64:128, h0 + 1, qc * QN:(qc + 1) * QN],
                                pqs[qc][64:128, :], qkb[64:128, jj:jj + 1])
                    yield

            def proj_units(j):
                # y^T tile j = wp^T @ attn, + proj bias on eviction
                pys = [ps_w.tile([P, QN], f32, tag="pw", name=f"py{qc}")
                       for qc in range(QC)]
                for k in range(CT):
                    for qc in range(QC):
                        nc.tensor.matmul(
                            pys[qc], wp[:, k, j * P:(j + 1) * P],
                            attn[:, k, qc * QN:(qc + 1) * QN],
                            start=(k == 0), stop=(k == CT - 1))
                    yield
                yT_dst = yT_d[:].rearrange("(j p) t -> p j t", p=P)
                for qc in range(QC):
                    yb = cst.tile([P, QN], f32, tag="yb", bufs=2, name="yb")
                    nc.vector.tensor_scalar_add(yb, pys[qc],
                                                pbias[:, j:j + 1])
                    nc.sync.dma_start(
                        yT_dst[:, j, qc * QN:(qc + 1) * QN], yb)
                yield

            def emit_chain(h, po_t):
                # denominators: inv = exp(-ln(s)) on ACT (ln+exp share
                # one table set; DVE reciprocal costs a fixed ~4us and the
                # custom-DVE approx op mis-executes on HW), broadcast across
                # 64 partitions via a DRAM round-trip DMA with a stride-0
                # partition source AP (gpsimd partition_broadcast mis-writes
                # on HW), then fused normalize+evict: attn = (po*1)*inv_bc
                inv_t = pt_pool.tile([1, QC, QN], f32, tag="inv", name="inv")
                nc.scalar.activation(inv_t, po_t[HD:HD + 1, :, :],
                                     AF.Ln, bias=0.0, scale=1.0)
                nc.scalar.activation(inv_t, inv_t,
                                     AF.Exp, bias=0.0, scale=-1.0)
                inv_bc = pt_pool.tile([HD, N], f32, tag="invbc", name="invbc")
                w = nc.sync.dma_start(inv_d[h], inv_t)
                r = nc.sync.dma_start(
                    inv_bc, bass.AP(tensor=inv_d, offset=h * N,
                                    ap=[[0, HD], [1, N]]))
                _add_dep_helper(r.ins, w.ins, sync=True, reason="inv RAW")
                pbase = (h % 2) * 64
                j = h // 2
                for qc in range(QC):
                    nc.vector.scalar_tensor_tensor(
                        attn[pbase:pbase + HD, j, qc * QN:(qc + 1) * QN],
                        po_t[0:HD, qc, :], 1.0,
                        inv_bc[0:HD, qc * QN:(qc + 1) * QN],
                        MUL, MUL)

            # ---- pre-block: all of V, then QK tile pair 0 ----
            for tt in range(KT):
                for _ in v_units(tt):
                    pass
            for _ in qk_units(0):
                pass

            # ---- sliding head pipeline ----
            # Per head h: the S/exp/bias-mult stream for h runs with the
            # previous head's PV matmuls and one unit of the QK projection
            # stream interleaved after every k-tile, so the PE queue always
            # has dependency-satisfied work (no bubbles, p-state stays up).
            stream = itertools.chain(*[qk_units(j) for j in range(1, NP)])
            po = {}
            pt = {}
            for h in range(NH):
                if h + 2 < NH:
                    load_bias(h + 2)
                pt[h] = pt_pool.tile([P, KT, N], f16, tag="pt",
                                     name=f"pt{h}")
                if h >= 1:
                    po[h - 1] = ps_o.tile([HD + 1, QC, QN], f32, tag="po",
                                          name=f"po{h - 1}")
                def pv_mms(hh, kt):
                    for qc in range(QC):
                        nc.tensor.matmul(
                            po[hh][:, qc, :], v_aug[:, kt, hh, :],
                            pt[hh][:, kt, qc * QN:(qc + 1) * QN],
                            start=(kt == 0), stop=(kt == KT - 1))

                for kt in range(KT):
                    pss = ps_s.tile([P, N], f32, tag="pss", name="pss",
                                    bufs=1)
                    for qc in range(QC):
                        nc.tensor.matmul(
                            pss[:, qc * QN:(qc + 1) * QN],
                            k_pad[:, h, kt * P:(kt + 1) * P],
                            q_t[:, h // 2, qc * QN:(qc + 1) * QN],
                            start=True, stop=True)
                    next(stream, None)
                    # PV fillers staggered one k-tile behind the S stream;
                    # the FIRST two pt tiles are bias-multiplied on the
                    # slower GPSIMD (their exps finish first, so the gpsimd
                    # queue runs them early with ~a full segment of slack)
                    if h >= 1 and kt >= 1:
                        pv_mms(h - 1, kt - 1)
                    nc.scalar.activation(pt[h][:, kt, :], pss,
                                         AF.Exp, bias=0.0, scale=1.0)
                    if kt in (0, 1) and h < NH - 1:
                        nc.gpsimd.tensor_mul(pt[h][:, kt, :], pt[h][:, kt, :],
                                             biasT[h][:, kt, :])
                    else:
                        nc.vector.tensor_mul(pt[h][:, kt, :], pt[h][:, kt, :],
                                             biasT[h][:, kt, :])
                if h >= 1:
                    pv_mms(h - 1, KT - 1)
                # chain for h-2 at segment end: its ACT ln/exp then run
                # after this segment's softmax exps instead of stalling in
                # front of them
                if h >= 2:
                    emit_chain(h - 2, po[h - 2])
                    del po[h - 2], pt[h - 2]

            # ---- tail: last PV, last chains, output projection ----
            emit_chain(NH - 2, po[NH - 2])
            po[NH - 1] = ps_o.tile([HD + 1, QC, QN], f32, tag="po",
                                   name=f"po{NH - 1}")
            for kt in range(KT):
                for qc in range(QC):
                    nc.tensor.matmul(
                        po[NH - 1][:, qc, :], v_aug[:, kt, NH - 1, :],
                        pt[NH - 1][:, kt, qc * QN:(qc + 1) * QN],
                        start=(kt == 0), stop=(kt == KT - 1))
            emit_chain(NH - 1, po[NH - 1])
            for j in range(CT):
                for _ in proj_units(j):
                    pass

    nc.compile()
    return nc


def _get_nc():
    if "nc" not in _CACHE:
        _CACHE["nc"] = _build()
    return _CACHE["nc"]


def prepare_inputs(x, qkv_w, q_bias, v_bias, proj_w, proj_b, rel_table,
                   rel_index):
    """Host-side resharding/layout prep. Returns per-core input maps."""
    scale = HD ** -0.5
    x = np.asarray(x, np.float32)
    qkv_w = np.asarray(qkv_w, np.float32)
    q_bias = np.asarray(q_bias, np.float32)
    v_bias = np.asarray(v_bias, np.float32)
    proj_w = np.asarray(proj_w, np.float32)
    proj_b = np.asarray(proj_b, np.float32)
    rel_table = np.asarray(rel_table, np.float32)
    rel_index = np.asarray(rel_index)

    wq = qkv_w[0:C, :] * scale          # [o, c] rows scaled
    wk = qkv_w[C:2 * C, :]
    wv_ = qkv_w[2 * C:3 * C, :]
    wqk = np.ascontiguousarray(
        np.concatenate([wq, wk], axis=0).T).astype(np.float16)   # [c, 2C]
    wv_t = np.ascontiguousarray(wv_.T).astype(np.float16)        # [c, C]
    wp = np.ascontiguousarray(proj_w.T).astype(np.float16)       # [c, co]
    qkb = np.concatenate([q_bias * scale, np.zeros(C, np.float32)])
    qkb = np.ascontiguousarray(qkb.reshape(OT_QK, P))
    pb = np.ascontiguousarray(proj_b.reshape(CT, P))

    # bias[q, k, h] = rel_table[rel_index[q, k]]; ship exp(biasT[h, k, q])
    # so the kernel folds the softmax bias multiplicatively into P^T
    bias = rel_table[rel_index.reshape(-1)].reshape(N, N, NH)
    biasT = np.ascontiguousarray(
        np.exp(bias.transpose(2, 1, 0), dtype=np.float32)).astype(np.float16)

    shared = {
        "wqk": wqk, "wv": wv_t, "wp": wp, "qkb": qkb,
        "vb": v_bias.astype(np.float16), "pb": pb, "biasT": biasT,
    }
    in_maps = []
    for b in range(B):
        xt = np.ascontiguousarray(
            x[b].reshape(N, C).T).astype(np.float16)
        in_maps.append({"xT": xt, **shared})
    return in_maps


def kernel(x, qkv_w, q_bias, v_bias, proj_w, proj_b, rel_table, rel_index,
           _trace=False):
    from concourse.bass_utils import run_bass_kernel_spmd

    nc = _get_nc()
    in_maps = prepare_inputs(x, qkv_w, q_bias, v_bias, proj_w, proj_b,
                             rel_table, rel_index)
    kwargs = {}
    if _trace:
        import concourse.bass_utils as _bu
        _bu.upload_artifacts = lambda tmpdir: tmpdir
        kwargs = {"trace": True}
    res = run_bass_kernel_spmd(nc, in_maps, core_ids=list(range(B)), **kwargs)
    out = np.empty((B, WS, WS, C), np.float32)
    for b in range(B):
        out[b] = res.results[b]["yT"].T.reshape(WS, WS, C)
    if _trace:
        _CACHE["last_result"] = res
    return out
